# revision 12
# baseline (speedup 1.0000x reference)
"""Trainium2 Bass kernel for AssetGATEncoder (2-layer GATv2, N=30000, E=480000).

Strategy (8 NeuronCores, SPMD):
- Nodes partitioned by DESTINATION: core c owns dst rows [c*3750, (c+1)*3750).
- Host sorts edges (with self-loops) by dst, buckets per core, groups them into
  128-dst blocks padded to a uniform edge count EB.
- Per layer, each core computes its shard of the packed projection table
  [xl | xr] = h @ [Wl | Wr] + b, AllGathers the full table (bf16), then per
  block: dma_gather xl[src] and xr[dst] rows, fused leaky-relu, att-dot via
  strided reduce, exp, and a masked-matmul (fp8 one-hot masks, resident in
  SBUF) that computes both the alpha-weighted message sums and the softmax
  denominators in one PE pass. Softmax division is applied per-dst afterwards.
- Layer epilogues (bias, LayerNorm, elu) run per 128-dst block; layer-1 tables
  are produced inside the layer-0 epilogue via PE transposes.
"""
import os
import numpy as np
import ml_dtypes

import concourse.bacc as bacc
import concourse.bass as bass
import concourse.mybir as mybir
import concourse.tile as tile
from concourse.bass_utils import run_bass_kernel_spmd

F32 = mybir.dt.float32
BF16 = mybir.dt.bfloat16
FP8 = mybir.dt.float8e4
I16 = mybir.dt.int16
AF = mybir.ActivationFunctionType
OP = mybir.AluOpType

N = 30000
NCORES = 8
NLOC = N // NCORES          # 3750 dst nodes per core
NBLK = (NLOC + 127) // 128  # 30 blocks of 128 dsts
F_IN = 128
HID = 64
HEADS = 4
FEAT0 = HEADS * HID         # 256
EMB = 32
SLOPE = 0.2
SM_EPS = 1e-16
LN_EPS = 1e-5
EXP_SHIFT = -3.0            # constant shift inside exp (cancels in softmax)

LAST_EXEC_NS = None
LAST_RESULT = None
bf = ml_dtypes.bfloat16
f8 = ml_dtypes.float8_e4m3


def _wrap_idx(idx, eb):
    """[eb] int -> [128, eb//16] int16 wrapped layout for dma_gather."""
    a = idx.reshape(eb // 16, 16).T.astype(np.int16)   # [16, eb/16]
    return np.tile(a, (8, 1))                          # [128, eb/16]


def _bcast(ap, extra):
    """Append zero-stride dims to an AP: extra = list of counts."""
    return bass.AP(tensor=ap.tensor, offset=ap.offset,
                   ap=[*ap.ap, *[[0, c] for c in extra]])


def _mid_bcast(ap, count):
    """[p, X] AP -> [p, count(bcast), X]."""
    return bass.AP(tensor=ap.tensor, offset=ap.offset,
                   ap=[ap.ap[0], [0, count], *ap.ap[1:]])


def _host_prep(x, edge_index, Wp, bp, Wl0, bl0, Wr0, br0, att0, bias0, g0, be0,
               Wl1, bl1, Wr1, br1, att1, bias1, g1, be1):
    src = np.asarray(edge_index[0], np.int64)
    dst = np.asarray(edge_index[1], np.int64)
    si = np.arange(N, dtype=np.int64)
    src = np.concatenate([src, si])
    dst = np.concatenate([dst, si])
    order = np.argsort(dst, kind="stable")
    src, dst = src[order], dst[order]

    # bucket edges per core / per 128-dst block
    per_core = []
    eb_max = 0
    for c in range(NCORES):
        lo, hi = c * NLOC, (c + 1) * NLOC
        m = (dst >= lo) & (dst < hi)
        s_c, d_c = src[m], dst[m] - lo
        blocks = []
        for b in range(NBLK):
            mb = (d_c >= b * 128) & (d_c < min((b + 1) * 128, NLOC))
            blocks.append((s_c[mb], d_c[mb] - b * 128))
            eb_max = max(eb_max, mb.sum())
        per_core.append(blocks)
    eb = int(np.ceil(eb_max / 128) * 128)
    nch = eb // 128

    sidx = np.zeros((NCORES, NBLK, 128, eb // 16), np.int16)
    didx = np.zeros((NCORES, NBLK, 128, eb // 16), np.int16)
    masks = np.zeros((NCORES, 128, NBLK * nch * 128), f8)
    for c in range(NCORES):
        for b in range(NBLK):
            s_b, dloc = per_core[c][b]
            ne = len(s_b)
            sp = np.zeros(eb, np.int64)
            dp = np.zeros(eb, np.int64)
            sp[:ne] = s_b
            dp[:ne] = c * NLOC + b * 128 + dloc   # global row in the packed table
            sidx[c, b] = _wrap_idx(sp, eb)
            didx[c, b] = _wrap_idx(dp, eb)
            mm = np.zeros((eb, 128), np.float32)
            mm[np.arange(ne), dloc] = 1.0
            # -> [128 edge-partitions, nch*128] per block
            mb = mm.reshape(nch, 128, 128).transpose(1, 0, 2).reshape(128, nch * 128)
            masks[c, :, b * nch * 128:(b + 1) * nch * 128] = mb.astype(f8)

    xT = np.ascontiguousarray(np.asarray(x, np.float32).T.astype(bf))  # [128, N]

    def t128(v, w):  # replicate a row vector to a [128, w] tile
        return np.tile(np.asarray(v, np.float32).reshape(1, w), (128, 1))

    consts = {
        "Wp": np.asarray(Wp, np.float32).astype(bf),                       # [128,64]
        "bp_t": t128(bp, HID).astype(np.float32),                          # [128,64]
        "Wpack0": np.concatenate([Wl0, Wr0], 1).astype(bf),                # [64,512]
        "bpk0_t": t128(np.concatenate([bl0, br0]), 2 * FEAT0).astype(bf),  # [128,512]
        "W1h": np.concatenate(                                             # [128,128]
            [np.concatenate([Wl1, Wr1], 1)[:128],
             np.concatenate([Wl1, Wr1], 1)[128:]], 1).astype(bf),
        "bpk1_t": t128(np.concatenate([bl1, br1]), 2 * EMB).astype(bf),    # [128,64]
        "att0_t": t128(np.asarray(att0).reshape(-1), FEAT0).astype(bf),    # [128,256]
        "att1_t": t128(np.asarray(att1).reshape(-1), EMB).astype(bf),      # [128,32]
        "bga0_t": t128(bias0, FEAT0).astype(np.float32),                   # [128,256]
        "bga1_t": t128(bias1, EMB).astype(np.float32),                     # [128,32]
        "g0_t": t128(g0, FEAT0).astype(np.float32),
        "be0_t": t128(be0, FEAT0).astype(np.float32),
        "g1_t": t128(g1, EMB).astype(np.float32),
        "be1_t": t128(be1, EMB).astype(np.float32),
        "ident": np.eye(128).astype(bf),
    }
    return xT, sidx, didx, masks, consts, eb, nch


def _build(eb, nch):
    TOTCH = NBLK * nch
    nc = bacc.Bacc("TRN2", target_bir_lowering=False)

    # ---- external inputs
    P = {}
    for name, shape, dt in [
        ("xT", [F_IN, NLOC], BF16),
        ("sidx", [NBLK, 128, eb // 16], I16),
        ("didx", [NBLK, 128, eb // 16], I16),
        ("masks", [128, TOTCH * 128], FP8),
        ("Wp", [F_IN, HID], BF16), ("bp_t", [128, HID], F32),
        ("Wpack0", [HID, 2 * FEAT0], BF16), ("bpk0_t", [128, 2 * FEAT0], BF16),
        ("W1h", [128, 2 * EMB * 2], BF16), ("bpk1_t", [128, 2 * EMB], BF16),
        ("att0_t", [128, FEAT0], BF16), ("att1_t", [128, EMB], BF16),
        ("bga0_t", [128, FEAT0], F32), ("bga1_t", [128, EMB], F32),
        ("g0_t", [128, FEAT0], F32), ("be0_t", [128, FEAT0], F32),
        ("g1_t", [128, EMB], F32), ("be1_t", [128, EMB], F32),
        ("ident", [128, 128], BF16),
    ]:
        P[name] = nc.declare_dram_parameter(name, shape, dt, isOutput=False)
    out_ext = nc.declare_dram_parameter("out", [NLOC, EMB], F32, isOutput=True)

    # ---- internal DRAM
    xlr0_loc = nc.dram_tensor("xlr0_loc", [NLOC, 2 * FEAT0], BF16)
    xlr0_full = nc.dram_tensor("xlr0_full", [N, 2 * FEAT0], BF16)
    xlr1_loc = nc.dram_tensor("xlr1_loc", [NLOC, 128], BF16)
    dump = nc.dram_tensor("dump", [128, 4096], BF16)
    xlr1_full = nc.dram_tensor("xlr1_full", [N, 128], BF16)

    rows_of = lambda b: min(128, NLOC - b * 128)

    with tile.TileContext(nc) as tc:
        with (
            tc.tile_pool(name="cst", bufs=1) as cst,
            tc.tile_pool(name="sb", bufs=1) as sb,
            tc.tile_pool(name="ps", bufs=1, space="PSUM") as ps,
        ):
            # ---- load constants
            C = {}
            for name in ["Wp", "bp_t", "Wpack0", "bpk0_t", "W1h", "bpk1_t",
                         "att0_t", "att1_t", "bga0_t", "bga1_t",
                         "g0_t", "be0_t", "g1_t", "be1_t", "ident"]:
                t = cst.tile(list(P[name].shape), P[name].dtype, tag=name)
                nc.sync.dma_start(t[:], P[name][:])
                C[name] = t
            mask_sb = cst.tile([128, TOTCH * 128], FP8, tag="masks")
            nc.sync.dma_start(mask_sb[:], P["masks"][:])

            def elu_f32(dst_ap, src_ap, shape, tag):
                """dst = elu(src); src f32-ish AP, dst any dtype."""
                r = sb.tile(shape, F32, tag=f"{tag}_r")
                m = sb.tile(shape, F32, tag=f"{tag}_m")
                ep = sb.tile(shape, F32, tag=f"{tag}_e")
                nc.vector.tensor_scalar_max(r[:], src_ap, 0.0)
                nc.vector.tensor_scalar_min(m[:], src_ap, 0.0)
                nc.scalar.activation(ep[:], m[:], AF.Exp)
                nc.vector.scalar_tensor_tensor(dst_ap, r[:], -1.0, ep[:],
                                               op0=OP.add, op1=OP.add)

            def layer_norm(dst_ap, src_ap, width, g_t, be_t, tag):
                """dst = LN(src) * g + be over `width` features (f32 in/out)."""
                mu = sb.tile([128, 1], F32, tag=f"{tag}_mu")
                xc = sb.tile([128, width], F32, tag=f"{tag}_xc")
                sq = sb.tile([128, width], F32, tag=f"{tag}_sq")
                var = sb.tile([128, 1], F32, tag=f"{tag}_v")
                st = sb.tile([128, 1], F32, tag=f"{tag}_s")
                nc.vector.tensor_reduce(mu[:], src_ap, axis=mybir.AxisListType.X,
                                        op=OP.add)
                nc.vector.tensor_scalar_mul(mu[:], mu[:], 1.0 / width)
                nc.vector.tensor_scalar_sub(xc[:], src_ap, mu[:])
                nc.vector.scalar_tensor_tensor(sq[:], xc[:], 0.0, xc[:],
                                               op0=OP.add, op1=OP.mult,
                                               accum_out=var[:])
                nc.vector.tensor_scalar(st[:], var[:], 1.0 / width, LN_EPS,
                                        op0=OP.mult, op1=OP.add)
                nc.scalar.activation(st[:], st[:], AF.Sqrt)
                nc.vector.reciprocal(st[:], st[:])
                nc.vector.tensor_scalar_mul(xc[:], xc[:], st[:])
                nc.vector.tensor_tensor(xc[:], xc[:], g_t[:], OP.mult)
                nc.vector.tensor_tensor(dst_ap, xc[:], be_t[:], OP.add)

            # ================= phase 1: layer-0 tables =================
            for ch in range((NLOC + 127) // 128):
                rows = rows_of(ch)
                xt = sb.tile([128, 128], BF16, tag="p1_xt")
                nc.sync.dma_start(xt[:, :rows], P["xT"][:, ch * 128:ch * 128 + rows])
                ph = ps.tile([128, HID], F32, tag="p1_h")
                nc.tensor.matmul(ph[:], xt[:], C["Wp"][:], start=True, stop=True)
                hb = sb.tile([128, HID], F32, tag="p1_hb")
                nc.vector.tensor_tensor(hb[:], ph[:], C["bp_t"][:], OP.add)
                h = sb.tile([128, HID], BF16, tag="p1_hbf")
                elu_f32(h[:], hb[:], [128, HID], "el")
                # hT via PE transpose ([128,64] -> [64,128])
                pt = ps.tile([HID, 128], BF16, tag="p1_pt")
                nc.tensor.transpose(pt[:], h[:], C["ident"][:])
                hT = sb.tile([HID, 128], BF16, tag="p1_hT")
                nc.vector.tensor_copy(hT[:], pt[:])
                ptab = ps.tile([128, 2 * FEAT0], F32, tag="p1_tab")
                nc.tensor.matmul(ptab[:], hT[:], C["Wpack0"][:], start=True, stop=True)
                tabb = sb.tile([128, 2 * FEAT0], BF16, tag="p1_tabb")
                nc.vector.tensor_tensor(tabb[:], ptab[:], C["bpk0_t"][:], OP.add)
                nc.sync.dma_start(xlr0_loc[ch * 128:ch * 128 + rows, :],
                                  tabb[:rows, :])

            # ================= phase 2: AllGather layer-0 table ========
            if int(os.environ.get("K_STAGE", "4")) >= 2:
                nc.gpsimd.collective_compute(
                    "AllGather", OP.bypass, replica_groups=[list(range(NCORES))],
                    ins=[xlr0_loc[:]], outs=[xlr0_full[:]])

            # ================= per-layer edge phase ====================
            LVL = int(os.environ.get("K_EDGE", "5"))

            def edge_layer(layer):
                if layer == 0:
                    feat, heads, width = FEAT0, HEADS, FEAT0
                    table, att_t = xlr0_full, C["att0_t"]
                    elem, estep, xr_off = FEAT0, 2 * FEAT0, FEAT0
                else:
                    feat, heads, width = EMB, 1, EMB
                    table, att_t = xlr1_full, C["att1_t"]
                    elem, estep, xr_off = 128, 128, 0
                hd = feat // heads  # per-head dim (64 / 32)

                for b in range(NBLK):
                    rows = rows_of(b)
                    tg = "E"
                    si = sb.tile([128, eb // 16], I16, tag=f"{tg}_si", bufs=2)
                    di = sb.tile([128, eb // 16], I16, tag=f"{tg}_di", bufs=2)
                    nc.sync.dma_start(si[:], P["sidx"][b])
                    nc.sync.dma_start(di[:], P["didx"][b])
                    xlg = sb.tile([128, nch, elem], BF16, tag=f"{tg}_xlg", bufs=2)
                    xrg = sb.tile([128, nch, elem], BF16, tag=f"{tg}_xrg", bufs=2)
                    NS = 1024
                    for sgi in range((eb + NS - 1) // NS):
                        n = min(NS, eb - sgi * NS)
                        c0, cn = sgi * (NS // 128), n // 128
                        nc.gpsimd.dma_gather(
                            xlg[:, c0:c0 + cn, :], table[:, 0:elem],
                            si[:, sgi * (NS // 16):sgi * (NS // 16) + n // 16],
                            n, n, elem, elem_step=estep)
                        nc.gpsimd.dma_gather(
                            xrg[:, c0:c0 + cn, :], table[:, xr_off:xr_off + elem],
                            di[:, sgi * (NS // 16):sgi * (NS // 16) + n // 16],
                            n, n, elem, elem_step=estep)
                    if LVL < 2:
                        if b == 0:
                            nc.sync.dma_start(dump[:, 0:elem], xlg[:, 0, :])
                            nc.sync.dma_start(dump[:, elem:2 * elem], xrg[:, 0, :])
                        continue
                    if layer == 0:
                        xl_ap, xr_ap = xlg[:], xrg[:]
                    else:
                        xl_ap = xlg[:, :, 0:EMB]
                        xr_ap = xrg[:, :, EMB:2 * EMB]
                    # u = xl + xr ; z = max(0.2u, u) ; zm = z*att
                    u = sb.tile([128, nch, width], BF16, tag=f"{tg}_u")
                    nc.vector.tensor_tensor(u[:], xl_ap, xr_ap, OP.add)
                    z = sb.tile([128, nch, width], BF16, tag=f"{tg}_z")
                    nc.vector.scalar_tensor_tensor(z[:], u[:], SLOPE, u[:],
                                                   op0=OP.mult, op1=OP.max)
                    zm = sb.tile([128, nch, width], BF16, tag=f"{tg}_zm")
                    nc.vector.tensor_tensor(zm[:], z[:],
                                            _mid_bcast(att_t[:], nch), OP.mult)
                    # e = per-head sum ; a = exp(e + EXP_SHIFT)
                    e = sb.tile([128, nch, heads], F32, tag=f"{tg}_e")
                    zm4 = zm[:].rearrange("p n (h k) -> p n h k", k=hd)
                    nc.vector.tensor_reduce(e[:], zm4, axis=mybir.AxisListType.X,
                                            op=OP.add)
                    a = sb.tile([128, nch, heads], BF16, tag=f"{tg}_a")
                    nc.scalar.activation(a[:], e[:], AF.Exp)
                    if LVL < 3:
                        if b == 0:
                            nc.sync.dma_start(dump[:, 0:nch * heads], a[:].rearrange("p n h -> p (n h)"))
                        continue
                    # wa = [a*xl | a]
                    wa = sb.tile([128, nch, width + heads], BF16, tag=f"{tg}_wa", bufs=2)
                    wa_x = wa[:, :, 0:width].rearrange("p n (h k) -> p n h k", k=hd)
                    a4 = a[:].rearrange("p n (h o) -> p n h o", o=1)
                    nc.vector.tensor_tensor(wa_x, xl_ap.rearrange(
                        "p n (h k) -> p n h k", k=hd), _bcast(a4, [hd])[:, :, :, 0],
                        OP.mult)
                    nc.vector.tensor_copy(wa[:, :, width:width + heads], a[:])
                    if LVL < 4:
                        if b == 0:
                            nc.sync.dma_start(dump[:, 0:nch * (width + heads)].rearrange("p (n w) -> p n w", w=width + heads), wa[:])
                        continue
                    # masked-matmul aggregation
                    po = ps.tile([128, width + heads], F32, tag=f"{tg}_po", bufs=2)
                    for ch in range(nch):
                        mk = mask_sb[:, (b * nch + ch) * 128:(b * nch + ch + 1) * 128]
                        nc.tensor.matmul(po[:], mk, wa[:, ch, :],
                                         start=(ch == 0), stop=(ch == nch - 1))
                    if LVL < 5:
                        if b == 0:
                            pocp = sb.tile([128, width + heads], BF16, tag="pocp")
                            nc.vector.tensor_copy(pocp[:], po[:])
                            nc.sync.dma_start(dump[:, 0:width + heads], pocp[:])
                        continue
                    # softmax divide
                    s = sb.tile([128, heads], F32, tag=f"{tg}_s")
                    nc.vector.tensor_scalar_add(s[:], po[:, width:width + heads],
                                                SM_EPS)
                    nc.vector.reciprocal(s[:], s[:])
                    onrm = sb.tile([128, width], F32, tag=f"{tg}_on")
                    if heads > 1:
                        s4 = s[:].rearrange("p (h o) -> p h o", o=1)
                        nc.vector.tensor_tensor(
                            onrm[:].rearrange("p (h k) -> p h k", k=hd),
                            po[:, 0:width].rearrange("p (h k) -> p h k", k=hd),
                            _bcast(s4, [hd])[:, :, 0], OP.mult)
                    else:
                        nc.vector.tensor_scalar_mul(onrm[:], po[:, 0:width], s[:])
                    # + gat bias, LN, (elu)
                    gb = C["bga0_t"] if layer == 0 else C["bga1_t"]
                    nc.vector.tensor_tensor(onrm[:], onrm[:], gb[:], OP.add)
                    lnout = sb.tile([128, width], F32, tag=f"{tg}_ln")
                    if layer == 0:
                        layer_norm(lnout[:], onrm[:], width, C["g0_t"],
                                   C["be0_t"], "ln")
                        h1 = sb.tile([128, width], BF16, tag=f"{tg}_h1")
                        elu_f32(h1[:], lnout[:], [128, width], "el")
                        # layer-1 table rows: transpose h1, matmul W1h halves
                        px = ps.tile([128, 2 * EMB], F32, tag=f"{tg}_px")
                        for half in range(2):
                            pt1 = ps.tile([128, 128], BF16, tag=f"{tg}_pt1")
                            nc.tensor.transpose(pt1[:], h1[:, half * 128:(half + 1) * 128],
                                                C["ident"][:])
                            hT1 = sb.tile([128, 128], BF16, tag=f"{tg}_hT1")
                            nc.vector.tensor_copy(hT1[:], pt1[:])
                            nc.tensor.matmul(px[:], hT1[:],
                                             C["W1h"][:, half * 2 * EMB:(half + 1) * 2 * EMB],
                                             start=(half == 0), stop=(half == 1))
                        tb1 = sb.tile([128, 2 * EMB], BF16, tag=f"{tg}_tb1")
                        nc.vector.tensor_tensor(tb1[:], px[:], C["bpk1_t"][:], OP.add)
                        nc.sync.dma_start(
                            xlr1_loc[b * 128:b * 128 + rows, 0:2 * EMB], tb1[:rows, :])
                    else:
                        L1EP = int(os.environ.get("K_L1EP", "3"))
                        if L1EP >= 2:
                            layer_norm(lnout[:], onrm[:], width, C["g1_t"],
                                       C["be1_t"], "ln")
                        else:
                            nc.vector.tensor_copy(lnout[:], onrm[:])
                        if L1EP >= 3:
                            nc.sync.dma_start(out_ext[b * 128:b * 128 + rows, :],
                                              lnout[:rows, :])
                        elif b == 0:
                            nc.sync.dma_start(out_ext[0:rows, :], lnout[:rows, :])

            stage = int(os.environ.get("K_STAGE", "4"))
            if stage >= 3:
                edge_layer(0)
            if stage >= 4:
                nc.gpsimd.collective_compute(
                    "AllGather", OP.bypass, replica_groups=[list(range(NCORES))],
                    ins=[xlr1_loc[:]], outs=[xlr1_full[:]])
                edge_layer(1)
            if stage < 4:
                # dummy out so the graph has its output: copy from xlr0_loc
                dbg = sb.tile([128, EMB], BF16, tag="dbg")
                dbgf = sb.tile([128, EMB], F32, tag="dbgf")
                for b in range(NBLK):
                    rows = rows_of(b)
                    nc.sync.dma_start(dbg[:rows, :], xlr0_loc[b * 128:b * 128 + rows, 0:EMB])
                    nc.vector.tensor_copy(dbgf[:rows, :], dbg[:rows, :])
                    nc.sync.dma_start(out_ext[b * 128:b * 128 + rows, :], dbgf[:rows, :])

    nc.compile()
    return nc


def kernel(**inputs):
    xT, sidx, didx, masks, consts, eb, nch = _host_prep(**inputs)
    nc = _build(eb, nch)
    in_maps = []
    for c in range(NCORES):
        m = {
            "xT": np.ascontiguousarray(xT[:, c * NLOC:(c + 1) * NLOC]),
            "sidx": sidx[c], "didx": didx[c], "masks": masks[c],
        }
        m.update(consts)
        in_maps.append(m)
    trace = bool(int(os.environ.get("K_TRACE", "0")))
    res = run_bass_kernel_spmd(nc, in_maps, list(range(NCORES)), trace=trace)
    global LAST_EXEC_NS, LAST_RESULT
    LAST_EXEC_NS = res.exec_time_ns
    LAST_RESULT = res
    out = np.concatenate([np.asarray(res.results[c]["out"]) for c in range(NCORES)], 0)
    return out.astype(np.float32)
